# revision 1
# baseline (speedup 1.0000x reference)
"""Trainium2 Bass kernel for nn_CrossAttention (B=4, Sx=Sy=2048, D=1024, f32).

Sharding: data-parallel over (batch b, query-half h) -> 8 cores; each core
computes full cross-attention for 1024 query rows of one batch against all
2048 keys of that batch (K/V projections duplicated across the 2 cores
sharing a batch; no collectives).

Per-core pipeline:
  P1: QT[e,s]  = (Wq'^T x xT) + bq'     (Wq' = Wq/sqrt(D), folded on host)
  P2: KT[e,t]  = (Wk^T x yT)  + bk      (scores-ready transposed layout)
  P3: V[t,e]   = yT^T x Wv              (bias bv folded on host, post-gather)
  P4 (per 512-wide s-superblock):
      PT[t,s]  = exp(KT^T @ QT)         (scores^T, softmax numerator)
      out[s,e] = (PT^T @ V) / (PT^T @ ones)

Numerics: projections and scores run in float32r (~TF32 precision at bf16
speed, fp32 PSUM accumulation); the final PV matmul runs in bf16 (softmax
probabilities tolerate it, and halving V/PT frees the SBUF needed to
prefetch every weight load under compute).

All three weight matrices stream through one 3-slot pool of 512-column
halves; slot cycling makes each load overlap the previous phase's compute.
"""

import numpy as np

import concourse.bacc as bacc
import concourse.bass as bass
import concourse.tile as tile
import concourse.mybir as mybir
from concourse.bass_utils import run_bass_kernel_spmd

F32 = mybir.dt.float32
F32R = mybir.dt.float32r
BF16 = mybir.dt.bfloat16

B, SX, SY, D = 4, 2048, 2048, 1024
NCORES = 8
SXH = SX // 2          # query rows per core
DB = D // 128          # contraction blocks
EB = D // 128          # output-feature blocks
TBLK = SY // 128       # key blocks of 128
SSB = 256            # s-superblock width
CH = 256               # activation staging chunk (tokens)

_CACHE = {}


def _build():
    nc = bacc.Bacc("TRN2", target_bir_lowering=False, debug=False,
                   num_devices=NCORES, dynamic_dma_scratch_size=2048)

    xt_d = nc.dram_tensor("xt", [DB, 128, SXH], F32R, kind="ExternalInput").ap()
    yt_d = nc.dram_tensor("yt", [DB, 128, SY], F32R, kind="ExternalInput").ap()
    wq_d = nc.dram_tensor("wq", [DB, 128, D], F32R, kind="ExternalInput").ap()
    wk_d = nc.dram_tensor("wk", [DB, 128, D], F32R, kind="ExternalInput").ap()
    wv_d = nc.dram_tensor("wv", [DB, 128, D], F32R, kind="ExternalInput").ap()
    bq_d = nc.dram_tensor("bq2", [EB, 128], F32, kind="ExternalInput").ap()
    bk_d = nc.dram_tensor("bk2", [EB, 128], F32, kind="ExternalInput").ap()
    out_d = nc.dram_tensor("out", [SXH, D], F32, kind="ExternalOutput").ap()

    with tile.TileContext(nc) as tc:
        with (
            tc.tile_pool(name="misc", bufs=1) as misc,
            tc.tile_pool(name="ostage", bufs=3) as ostage,
            tc.tile_pool(name="rstage", bufs=2) as rstage,
            tc.tile_pool(name="ps_big", bufs=5, space="PSUM") as ps_big,
            tc.tile_pool(name="ps_sum", bufs=2, space="PSUM") as ps_sum,
            tc.tile_pool(name="persist", bufs=1) as persist,
            tc.tile_pool(name="wst", bufs=3) as wst,
            tc.tile_pool(name="ast", bufs=3) as ast,
        ):
            bq_t = misc.tile([128, EB], F32)
            bk_t = misc.tile([128, EB], F32)
            ones_f = misc.tile([128, 2], F32)
            ones_t = misc.tile([128, 2], BF16)
            nc.vector.memset(ones_f, 1.0)
            nc.vector.tensor_copy(out=ones_t, in_=ones_f)

            QT = persist.tile([128, EB, SXH], F32R)   # 32 KB/part
            KT = persist.tile([128, EB, SY], F32R)    # 64 KB/part
            V = persist.tile([128, TBLK, D], BF16)    # 32 KB/part
            PT = persist.tile([128, TBLK, SSB], BF16)  # 16 KB/part (per-ssb)

            def w_half(w_dram, eh, pieces=1):
                wt = wst.tile([128, DB, 512], F32R, tag="w")
                step = 512 // pieces
                for q in range(pieces):
                    lo = q * step
                    nc.sync.dma_start(
                        out=wt[:, :, lo:lo + step],
                        in_=w_dram[:, :, eh * 512 + lo:eh * 512 + lo + step]
                        .rearrange("db p e -> p db e"))
                return wt

            # ---- P1: QT[e, s] = Wq'^T @ xT + bq' ----
            # first x-chunk loads ahead of the weights so the DMA queue
            # delivers the first matmul's operands as early as possible
            xtc0 = ast.tile([128, DB, CH], F32R, tag="a")
            nc.sync.dma_start(
                out=xtc0,
                in_=xt_d[:, :, 0:CH].rearrange("db p s -> p db s"))
            wq_h = [w_half(wq_d, 0, pieces=4), w_half(wq_d, 1)]
            nc.sync.dma_start(out=bq_t, in_=bq_d.rearrange("eb p -> p eb"))
            nc.sync.dma_start(out=bk_t, in_=bk_d.rearrange("eb p -> p eb"))
            for ci in range(SXH // CH):
                s0 = ci * CH
                if ci == 0:
                    xtc = xtc0
                else:
                    xtc = ast.tile([128, DB, CH], F32R, tag="a")
                    nc.sync.dma_start(
                        out=xtc,
                        in_=xt_d[:, :, s0:s0 + CH].rearrange("db p s -> p db s"))
                for eb in range(EB):
                    ps = ps_big.tile([128, CH], F32, tag="ps")
                    for db in range(DB):
                        nc.tensor.matmul(
                            ps,
                            lhsT=wq_h[eb // 4][:, db,
                                               (eb % 4) * 128:(eb % 4 + 1) * 128],
                            rhs=xtc[:, db, :],
                            start=(db == 0), stop=(db == DB - 1))
                    nc.vector.tensor_scalar_add(
                        out=QT[:, eb, s0:s0 + CH], in0=ps,
                        scalar1=bq_t[:, eb:eb + 1])

            # ---- P2: KT[e, t] = Wk^T @ yT + bk ----
            ytc0 = ast.tile([128, DB, CH], F32R, tag="a")
            nc.sync.dma_start(
                out=ytc0,
                in_=yt_d[:, :, 0:CH].rearrange("db p t -> p db t"))
            wk_h = [w_half(wk_d, 0), w_half(wk_d, 1)]
            for ci in range(SY // CH):
                t0 = ci * CH
                if ci == 0:
                    ytc = ytc0
                else:
                    ytc = ast.tile([128, DB, CH], F32R, tag="a")
                    nc.sync.dma_start(
                        out=ytc,
                        in_=yt_d[:, :, t0:t0 + CH].rearrange("db p t -> p db t"))
                eb_order = (list(range(4, 8)) + list(range(4))
                            if ci == SY // CH - 1 else list(range(EB)))
                for eb in eb_order:
                    ps = ps_big.tile([128, CH], F32, tag="ps")
                    for db in range(DB):
                        nc.tensor.matmul(
                            ps,
                            lhsT=wk_h[eb // 4][:, db,
                                               (eb % 4) * 128:(eb % 4 + 1) * 128],
                            rhs=ytc[:, db, :],
                            start=(db == 0), stop=(db == DB - 1))
                    nc.vector.tensor_scalar_add(
                        out=KT[:, eb, t0:t0 + CH], in0=ps,
                        scalar1=bk_t[:, eb:eb + 1])

            # ---- P3: V[t, e] = yT^T @ Wv ----
            ytc20 = ast.tile([128, DB, CH], F32R, tag="a")
            nc.sync.dma_start(
                out=ytc20,
                in_=yt_d[:, :, 0:CH].rearrange("db p t -> p db t"))
            wv_h = [w_half(wv_d, 0), w_half(wv_d, 1)]
            for ci in range(SY // CH):
                t0 = ci * CH
                if ci == 0:
                    ytc2 = ytc20
                else:
                    ytc2 = ast.tile([128, DB, CH], F32R, tag="a")
                    nc.sync.dma_start(
                        out=ytc2,
                        in_=yt_d[:, :, t0:t0 + CH].rearrange("db p t -> p db t"))
                ord_pairs = ([(tbi, eh) for eh in range(D // 512)
                              for tbi in range(CH // 128)] if ci == 0 else
                             [(tbi, eh) for tbi in range(CH // 128)
                              for eh in range(D // 512)])
                for tbi, eh in ord_pairs:
                    tb = ci * (CH // 128) + tbi
                    if True:
                        ps = ps_big.tile([128, 512], F32, tag="ps")
                        for db in range(DB):
                            nc.tensor.matmul(
                                ps,
                                lhsT=ytc2[:, db, tbi * 128:(tbi + 1) * 128],
                                rhs=wv_h[eh][:, db, :],
                                start=(db == 0), stop=(db == DB - 1))
                        nc.vector.tensor_copy(
                            out=V[:, tb, eh * 512:(eh + 1) * 512], in_=ps)

            # ---- P4: attention per s-superblock ----
            for ssb in range(SXH // SSB):
                s0 = ssb * SSB
                for tb in range(TBLK):
                    ps = ps_big.tile([128, SSB], F32, tag="ps")
                    for eb in range(EB):
                        nc.tensor.matmul(
                            ps,
                            lhsT=KT[:, eb, tb * 128:(tb + 1) * 128],
                            rhs=QT[:, eb, s0:s0 + SSB],
                            start=(eb == 0), stop=(eb == EB - 1))
                    nc.scalar.activation(
                        out=PT[:, tb, :], in_=ps,
                        func=mybir.ActivationFunctionType.Exp)
                for sbi in range(SSB // 128):
                    sl = sbi * 128
                    ps0 = ps_big.tile([128, 512], F32, tag="ps")
                    ps1 = ps_big.tile([128, 512], F32, tag="ps")
                    pss = ps_sum.tile([128, 2], F32, tag="pss")
                    for tb in range(TBLK):
                        lhsT = PT[:, tb, sl:sl + 128]
                        nc.tensor.matmul(
                            ps0, lhsT=lhsT, rhs=V[:, tb, 0:512],
                            start=(tb == 0), stop=(tb == TBLK - 1))
                        nc.tensor.matmul(
                            ps1, lhsT=lhsT, rhs=V[:, tb, 512:1024],
                            start=(tb == 0), stop=(tb == TBLK - 1))
                        nc.tensor.matmul(
                            pss, lhsT=lhsT, rhs=ones_t,
                            start=(tb == 0), stop=(tb == TBLK - 1))
                    rec = rstage.tile([128, 1], F32, tag="rec")
                    nc.vector.reciprocal(rec, pss[:, 0:1])
                    for eh in range(2):
                        o = ostage.tile([128, 512], F32, tag="o")
                        nc.vector.tensor_scalar_mul(
                            out=o, in0=(ps0 if eh == 0 else ps1),
                            scalar1=rec[:, 0:1])
                        nc.sync.dma_start(
                            out=out_d[s0 + sl:s0 + sl + 128,
                                      eh * 512:(eh + 1) * 512],
                            in_=o)

    nc.compile()
    return nc


def _get_nc():
    if "nc" not in _CACHE:
        _CACHE["nc"] = _build()
    return _CACHE["nc"]


def make_in_maps(x, y, Wq, bq, Wk, bk, Wv, bv):
    x = np.asarray(x, dtype=np.float32)
    y = np.asarray(y, dtype=np.float32)
    s = np.float32(1.0 / np.sqrt(D))
    wq = np.ascontiguousarray(
        (np.asarray(Wq, dtype=np.float32) * s).reshape(DB, 128, D))
    wk = np.ascontiguousarray(
        np.asarray(Wk, dtype=np.float32).reshape(DB, 128, D))
    wv = np.ascontiguousarray(
        np.asarray(Wv, dtype=np.float32).reshape(DB, 128, D))
    bq2 = np.ascontiguousarray(
        (np.asarray(bq, dtype=np.float32) * s).reshape(EB, 128))
    bk2 = np.ascontiguousarray(
        np.asarray(bk, dtype=np.float32).reshape(EB, 128))

    in_maps = []
    for c in range(NCORES):
        b, h = divmod(c, 2)
        xt = np.ascontiguousarray(
            x[b, h * SXH:(h + 1) * SXH, :].T).reshape(DB, 128, SXH)
        yt = np.ascontiguousarray(y[b].T).reshape(DB, 128, SY)
        in_maps.append({
            "xt": xt, "yt": yt, "wq": wq, "wk": wk, "wv": wv,
            "bq2": bq2, "bk2": bk2,
        })
    return in_maps


def assemble(results, bv):
    bv = np.asarray(bv, dtype=np.float32)
    out = np.empty((B, SX, D), dtype=np.float32)
    for c in range(NCORES):
        b, h = divmod(c, 2)
        out[b, h * SXH:(h + 1) * SXH, :] = results[c]["out"]
    out += bv[None, None, :]
    return out


def kernel(x, y, Wq, bq, Wk, bk, Wv, bv):
    nc = _get_nc()
    in_maps = make_in_maps(x, y, Wq, bq, Wk, bk, Wv, bv)
    res = run_bass_kernel_spmd(nc, in_maps, list(range(NCORES)))
    return assemble(res.results, bv)



# revision 2
# speedup vs baseline: 1.7909x; 1.7909x over previous
"""Trainium2 Bass kernel for nn_CrossAttention (B=4, Sx=Sy=2048, D=1024, f32).

Sharding: data-parallel over (batch b, query-half h) -> 8 cores; each core
computes full cross-attention for 1024 query rows of one batch against all
2048 keys of that batch. No collectives.

Algorithmic restructure vs the straightforward 5-matmul pipeline:
  scores = (x Wq)(y Wk)^T = x (Wq Wk^T) y^T   -> M := Wq Wk^T on HOST,
     eliminating the K projection entirely (A := x M, scores = A y^T).
  out = P (y Wv) = (P y) Wv                   -> N := P y on device,
     replacing {V projection (big) + P V (big)} by {P y (big) + N Wv (small)}.
  The bq-dependent logit term (bq.K^T, per-key) is host-computed and rides
  the exp's per-partition bias AP; the Q.bk term is constant per query row
  and cancels in softmax; bv is added on host after the gather.

All matmuls run in fp8 (e4m3) with MatmulPerfMode.DoubleRow (2 K-tiles per
instruction), using an error-compensated 3-term form per GEMM:
  X*W ~= Xhi*Whi + Xlo*Whi + Xhi*Wlo
with hi/lo e4m3 splits (host-side for inputs/weights, on-device for the
A/P/N intermediates: hi = ACT copy/exp from PSUM, lo = one fused DVE
scalar_tensor_tensor (psum*scale - hi)). End-to-end rel err ~3e-3.

Per-core Tensor-engine work drops from ~592k PE cycles (f32r pipeline) to
~297k cycles: A (49k) + scores (98k) + N=P y (98k) + N Wv (49k) + Z (tiny).

Scales (per core): psA = x*(32M) = 32A; A=AH+AL (e4m3 pair, ACT scale 1/32);
psS = A y^T = 32*z; P = exp(psS/32 + cvec/32 - 3) = PH+PL; psN = P y;
N/4 = NH+NL (ACT scale 1/4); psO = (N/4)(32Wv) = 8*P y Wv;
out = psO * 0.125 / Z with Z = ones-matmul over (PH+PL).
"""

import numpy as np
import ml_dtypes

import concourse.bacc as bacc
import concourse.bass as bass
import concourse.tile as tile
import concourse.mybir as mybir
from concourse.bass_utils import run_bass_kernel_spmd

F32 = mybir.dt.float32
E4 = mybir.dt.float8e4
E4NP = ml_dtypes.float8_e4m3
ALU = mybir.AluOpType
DR = mybir.MatmulPerfMode.DoubleRow
AF = mybir.ActivationFunctionType

B, SX, SY, D = 4, 2048, 2048, 1024
NCORES = 8
SXH = SX // 2          # query rows per core
DB = D // 128          # d (and d') ktiles
TB = SY // 128         # t ktiles
SBLK = 512             # s superblock width
NSB = SXH // SBLK      # superblocks per core
SHIFT = 3.0            # exp shift, cancels in normalization

_CACHE = {}


def _build():
    nc = bacc.Bacc("TRN2", target_bir_lowering=False, debug=False,
                   num_devices=NCORES, dynamic_dma_scratch_size=2048)

    xt8_d = nc.dram_tensor("xt8", [DB, 128, SXH], E4, kind="ExternalInput").ap()
    xtr_d = nc.dram_tensor("xtr", [DB, 128, SXH], E4, kind="ExternalInput").ap()
    mq8_d = nc.dram_tensor("mq8", [DB, 128, D], E4, kind="ExternalInput").ap()
    mql_d = nc.dram_tensor("mql", [DB, 128, D], E4, kind="ExternalInput").ap()
    yt8_d = nc.dram_tensor("yt8", [DB, 128, SY], E4, kind="ExternalInput").ap()
    ytr_d = nc.dram_tensor("ytr", [DB, 128, SY], E4, kind="ExternalInput").ap()
    yn8_d = nc.dram_tensor("yn8", [TB, 128, D], E4, kind="ExternalInput").ap()
    ynr_d = nc.dram_tensor("ynr", [TB, 128, D], E4, kind="ExternalInput").ap()
    wv8_d = nc.dram_tensor("wv8", [DB, 128, D], E4, kind="ExternalInput").ap()
    wvl_d = nc.dram_tensor("wvl", [DB, 128, D], E4, kind="ExternalInput").ap()
    cb_d = nc.dram_tensor("cb", [TB, 128], F32, kind="ExternalInput").ap()
    out_d = nc.dram_tensor("out", [SXH, D], F32, kind="ExternalOutput").ap()

    with tile.TileContext(nc) as tc:
        with (
            tc.tile_pool(name="persist", bufs=1) as persist,
            tc.tile_pool(name="work", bufs=1) as work,
            tc.tile_pool(name="ost", bufs=3) as ostp,
            tc.tile_pool(name="pfs", bufs=4) as pfs,
            tc.tile_pool(name="zrp", bufs=2) as zrp,
            tc.tile_pool(name="ps", bufs=6, space="PSUM") as psp,
            tc.tile_pool(name="psz", bufs=2, space="PSUM") as pszp,
        ):
            XT8 = persist.tile([128, DB, SXH], E4)
            MQ8 = persist.tile([128, DB, D], E4)
            MQL = persist.tile([128, DB, D], E4)
            XTR = persist.tile([128, DB, SXH], E4)
            CB = persist.tile([128, TB], F32)
            YT8 = persist.tile([128, DB, SY], E4)
            YTR = persist.tile([128, DB, SY], E4)
            YN8 = persist.tile([128, TB, D], E4)
            YNR = persist.tile([128, TB, D], E4)
            WV8 = persist.tile([128, DB, D], E4)
            WVL = persist.tile([128, DB, D], E4)
            ones = persist.tile([128, 2, 2], E4)

            # first-needed first: phase A operands, then B, C, D operands
            nc.sync.dma_start(out=XT8, in_=xt8_d.rearrange("k p s -> p k s"))
            nc.sync.dma_start(out=MQ8, in_=mq8_d.rearrange("k p d -> p k d"))
            nc.sync.dma_start(out=MQL, in_=mql_d.rearrange("k p d -> p k d"))
            nc.sync.dma_start(out=XTR, in_=xtr_d.rearrange("k p s -> p k s"))
            nc.vector.memset(ones, 1.0)
            nc.sync.dma_start(out=CB, in_=cb_d.rearrange("k p -> p k"))
            nc.sync.dma_start(out=YT8, in_=yt8_d.rearrange("k p t -> p k t"))
            nc.sync.dma_start(out=YTR, in_=ytr_d.rearrange("k p t -> p k t"))
            nc.sync.dma_start(out=YN8, in_=yn8_d.rearrange("k p d -> p k d"))
            nc.sync.dma_start(out=YNR, in_=ynr_d.rearrange("k p d -> p k d"))
            nc.sync.dma_start(out=WV8, in_=wv8_d.rearrange("k p d -> p k d"))
            nc.sync.dma_start(out=WVL, in_=wvl_d.rearrange("k p d -> p k d"))

            AH = work.tile([128, DB, SBLK], E4)
            AL = work.tile([128, DB, SBLK], E4)
            PH = work.tile([128, TB, SBLK], E4)
            PL = work.tile([128, TB, SBLK], E4)
            NH = work.tile([128, DB, SBLK], E4)
            NL = work.tile([128, DB, SBLK], E4)

            for sblk in range(NSB):
                s0 = sblk * SBLK

                # ---- Phase A: psA[d-blk, s] = x*(32M) = 32*A, 3-term ----
                for dblk in range(DB):
                    ps = psp.tile([128, SBLK], F32, tag="ps")
                    n = 0
                    for dp in range(DB // 2):
                        k = 2 * dp
                        for lhs, rhs in ((MQ8, XT8), (MQ8, XTR), (MQL, XT8)):
                            nc.tensor.matmul(
                                ps,
                                lhsT=lhs[:, k:k + 2, dblk * 128:(dblk + 1) * 128],
                                rhs=rhs[:, k:k + 2, s0:s0 + SBLK],
                                start=(n == 0), stop=(n == 3 * DB // 2 - 1),
                                perf_mode=DR)
                            n += 1
                    nc.scalar.activation(out=AH[:, dblk, :], in_=ps,
                                         func=AF.Copy, scale=1.0 / 32.0)
                    nc.vector.scalar_tensor_tensor(
                        out=AL[:, dblk, :], in0=ps, scalar=1.0 / 32.0,
                        in1=AH[:, dblk, :], op0=ALU.mult, op1=ALU.subtract)

                # ---- Phase B: psS[t-blk, s] = A y^T = 32*z, 3-term ----
                for tb in range(TB):
                    ps = psp.tile([128, SBLK], F32, tag="ps")
                    n = 0
                    for dp in range(DB // 2):
                        k = 2 * dp
                        for lhs, rhs in ((YT8, AH), (YT8, AL), (YTR, AH)):
                            nc.tensor.matmul(
                                ps,
                                lhsT=lhs[:, k:k + 2, tb * 128:(tb + 1) * 128],
                                rhs=rhs[:, k:k + 2, :],
                                start=(n == 0), stop=(n == 3 * DB // 2 - 1),
                                perf_mode=DR)
                            n += 1
                    pf = pfs.tile([128, SBLK], F32, tag="pf")
                    nc.scalar.activation(out=pf, in_=ps, func=AF.Exp,
                                         scale=1.0 / 32.0, bias=CB[:, tb:tb + 1])
                    nc.gpsimd.tensor_copy(out=PH[:, tb, :], in_=pf)
                    nc.vector.tensor_tensor(out=PL[:, tb, :], in0=pf,
                                            in1=PH[:, tb, :], op=ALU.subtract)

                # ---- Phase C: psN[d-blk, s] = P y, 3-term ----
                for dblk in range(DB):
                    ps = psp.tile([128, SBLK], F32, tag="ps")
                    n = 0
                    for tp in range(TB // 2):
                        k = 2 * tp
                        for lhs, rhs in ((YN8, PH), (YN8, PL), (YNR, PH)):
                            nc.tensor.matmul(
                                ps,
                                lhsT=lhs[:, k:k + 2, dblk * 128:(dblk + 1) * 128],
                                rhs=rhs[:, k:k + 2, :],
                                start=(n == 0), stop=(n == 3 * TB // 2 - 1),
                                perf_mode=DR)
                            n += 1
                    nc.scalar.activation(out=NH[:, dblk, :], in_=ps,
                                         func=AF.Copy, scale=0.25)
                    nc.vector.scalar_tensor_tensor(
                        out=NL[:, dblk, :], in0=ps, scalar=0.25,
                        in1=NH[:, dblk, :], op0=ALU.mult, op1=ALU.subtract)

                # ---- Z[s] = sum_t (PH+PL), per 128-query block ----
                zrecs = []
                for sb in range(SBLK // 128):
                    pz = pszp.tile([128, 2], F32, tag="pz")
                    n = 0
                    for lhs in (PH, PL):
                        for tp in range(TB // 2):
                            k = 2 * tp
                            nc.tensor.matmul(
                                pz,
                                lhsT=lhs[:, k:k + 2, sb * 128:(sb + 1) * 128],
                                rhs=ones,
                                start=(n == 0), stop=(n == TB - 1),
                                perf_mode=DR)
                            n += 1
                    zr = zrp.tile([128, 1], F32, tag="zr")
                    nc.vector.reciprocal(zr, pz[:, 0:1])
                    zrecs.append(zr)

                # ---- Phase D: psO[s-128, e] = (N/4)(32Wv) = 8*P y Wv ----
                for sb in range(SBLK // 128):
                    for eh in range(D // 512):
                        ps = psp.tile([128, 512], F32, tag="ps")
                        n = 0
                        for dp in range(DB // 2):
                            k = 2 * dp
                            for lhs, rhs in ((NH, WV8), (NL, WV8), (NH, WVL)):
                                nc.tensor.matmul(
                                    ps,
                                    lhsT=lhs[:, k:k + 2, sb * 128:(sb + 1) * 128],
                                    rhs=rhs[:, k:k + 2, eh * 512:(eh + 1) * 512],
                                    start=(n == 0), stop=(n == 3 * DB // 2 - 1),
                                    perf_mode=DR)
                                n += 1
                        o = ostp.tile([128, 512], F32, tag="o")
                        nc.vector.tensor_scalar(
                            out=o, in0=ps, scalar1=zrecs[sb][:, 0:1],
                            scalar2=0.125, op0=ALU.mult, op1=ALU.mult)
                        nc.sync.dma_start(
                            out=out_d[s0 + sb * 128:s0 + (sb + 1) * 128,
                                      eh * 512:(eh + 1) * 512],
                            in_=o)

    nc.compile()
    return nc


def _get_nc():
    if "nc" not in _CACHE:
        _CACHE["nc"] = _build()
    return _CACHE["nc"]


def _split(a):
    hi = np.asarray(a, dtype=np.float32).astype(E4NP)
    lo = (np.asarray(a, dtype=np.float32) - hi.astype(np.float32)).astype(E4NP)
    return hi, lo


def make_in_maps(x, y, Wq, bq, Wk, bk, Wv, bv):
    x = np.asarray(x, dtype=np.float32)
    y = np.asarray(y, dtype=np.float32)
    Wq = np.asarray(Wq, dtype=np.float32)
    Wk = np.asarray(Wk, dtype=np.float32)
    Wv = np.asarray(Wv, dtype=np.float32)
    bq = np.asarray(bq, dtype=np.float32)
    bk = np.asarray(bk, dtype=np.float32)

    M = (Wq.astype(np.float64) @ Wk.T.astype(np.float64)).astype(np.float32)
    mq8, mql = _split(32.0 * M)                     # [d', d]
    wv8, wvl = _split(32.0 * Wv)                    # [d, e]
    mq8 = np.ascontiguousarray(mq8.reshape(DB, 128, D))
    mql = np.ascontiguousarray(mql.reshape(DB, 128, D))
    wv8 = np.ascontiguousarray(wv8.reshape(DB, 128, D))
    wvl = np.ascontiguousarray(wvl.reshape(DB, 128, D))

    bqk = bq @ Wk.T                                 # [d]
    bqbk = float(bq @ bk)

    per_batch = []
    for b in range(B):
        yb = y[b]
        y8, yr = _split(yb)                          # [t, d]
        ybT = np.ascontiguousarray(yb.T)
        yt8, ytr = _split(ybT)                       # [d, t]
        cvec = (bqk @ yb.T) + bqbk                   # [t]
        cb = (cvec / 32.0 - SHIFT).astype(np.float32).reshape(TB, 128)
        per_batch.append({
            "yn8": np.ascontiguousarray(y8.reshape(TB, 128, D)),
            "ynr": np.ascontiguousarray(yr.reshape(TB, 128, D)),
            "yt8": np.ascontiguousarray(yt8.reshape(DB, 128, SY)),
            "ytr": np.ascontiguousarray(ytr.reshape(DB, 128, SY)),
            "cb": np.ascontiguousarray(cb),
        })

    in_maps = []
    for c in range(NCORES):
        b, h = divmod(c, 2)
        xbT = np.ascontiguousarray(x[b, h * SXH:(h + 1) * SXH, :].T)  # [d, s]
        xt8, xtr = _split(xbT)
        m = {
            "xt8": np.ascontiguousarray(xt8.reshape(DB, 128, SXH)),
            "xtr": np.ascontiguousarray(xtr.reshape(DB, 128, SXH)),
            "mq8": mq8, "mql": mql, "wv8": wv8, "wvl": wvl,
        }
        m.update(per_batch[b])
        in_maps.append(m)
    return in_maps


def assemble(results, bv):
    bv = np.asarray(bv, dtype=np.float32)
    out = np.empty((B, SX, D), dtype=np.float32)
    for c in range(NCORES):
        b, h = divmod(c, 2)
        out[b, h * SXH:(h + 1) * SXH, :] = results[c]["out"]
    out += bv[None, None, :]
    return out


def kernel(x, y, Wq, bq, Wk, bk, Wv, bv):
    nc = _get_nc()
    in_maps = make_in_maps(x, y, Wq, bq, Wk, bk, Wv, bv)
    res = run_bass_kernel_spmd(nc, in_maps, list(range(NCORES)))
    return assemble(res.results, bv)
